# revision 5
# baseline (speedup 1.0000x reference)
"""Trainium2 Bass kernel for nn_IntraAgg (GraphSAGE-style intra-aggregation).

Strategy: data-parallel over the 8192 seed nodes across 8 NeuronCores
(1024 seeds/core); feat_table and weights replicated. Per core:
  - indirect-DMA gather of the 32 neighbor rows per seed (2KB rows)
  - neighbor-sum via strided VectorE reduce
  - gen / env matmuls on TensorE (activations transposed on-chip via PE)
  - BatchNorm stats via per-core partial sums + 12KB AllReduce
  - final 1536x1536 matmul + relu
All fp32.
"""
import sys
sys.path.insert(0, '/opt/trn_rl_repo')

import numpy as np

import concourse.bass as bass
import concourse.mybir as mybir
from concourse import bacc
from concourse.tile import TileContext
from concourse.masks import make_identity

F32 = mybir.dt.float32
I32 = mybir.dt.int32
AF = mybir.ActivationFunctionType
ALU = mybir.AluOpType
AX = mybir.AxisListType

N_CORES = 8
N_NODES = 100000
F = 512            # feat dim
K = 32             # neighbors (col 0 = self)
B = 8192           # global batch
BL = B // N_CORES  # 1024 seeds per core
P = 128
NBLK = BL // P     # 8 seed-blocks per core
BN_EPS = 1e-5
KSUB = 8           # indices per sub-gather
NSUB = K // KSUB   # 4 sub-gathers per block

_CACHE = {}


def _build():
    nc = bacc.Bacc("TRN2", target_bir_lowering=False, debug=False,
                   num_devices=N_CORES)

    table = nc.declare_dram_parameter("feat_table", [N_NODES, F], F32, isOutput=False)
    wgen = nc.declare_dram_parameter("W_gen", [2 * F, 2 * F], F32, isOutput=False)
    w1 = nc.declare_dram_parameter("weight1", [3 * F, 3 * F], F32, isOutput=False)
    gamma = nc.declare_dram_parameter("gamma", [3 * F], F32, isOutput=False)
    beta = nc.declare_dram_parameter("beta", [3 * F], F32, isOutput=False)
    nidx = nc.declare_dram_parameter("neigh_idx", [BL, K], I32, isOutput=False)

    agg_o = nc.declare_dram_parameter("agg", [BL, F], F32, isOutput=True)
    to_o = nc.declare_dram_parameter("to", [BL, 3 * F], F32, isOutput=True)
    gen_o = nc.declare_dram_parameter("gen", [BL, 2 * F], F32, isOutput=True)
    raw_o = nc.declare_dram_parameter("raw", [BL, 2 * F], F32, isOutput=True)
    egen_o = nc.declare_dram_parameter("egen", [BL, 2 * F], F32, isOutput=True)
    eraw_o = nc.declare_dram_parameter("eraw", [BL, 2 * F], F32, isOutput=True)

    cc_in = nc.dram_tensor("cc_in", [P, 24], F32)
    cc_out = nc.dram_tensor("cc_out", [P, 24], F32, addr_space="Shared")

    # blocked views: seed s = blk*128 + p
    def blocked(h, nch):
        return h.ap().rearrange("(b p) c -> p b c", p=P)

    agg_v = blocked(agg_o, F)
    to_v = blocked(to_o, 3 * F)
    gen_v = blocked(gen_o, 2 * F)
    raw_v = blocked(raw_o, 2 * F)
    egen_v = blocked(egen_o, 2 * F)
    eraw_v = blocked(eraw_o, 2 * F)

    with TileContext(nc) as tc:
        with tc.tile_pool(name="const", bufs=1) as cpool, \
             tc.tile_pool(name="gath", bufs=2) as gpool, \
             tc.tile_pool(name="vec", bufs=2) as vpool, \
             tc.tile_pool(name="cat", bufs=2) as catpool, \
             tc.tile_pool(name="ft", bufs=NBLK) as fpool, \
             tc.tile_pool(name="ost", bufs=4) as opool, \
             tc.tile_pool(name="genp", bufs=2) as genpool, \
             tc.tile_pool(name="stat", bufs=2) as stpool, \
             tc.tile_pool(name="sq", bufs=1) as sqpool, \
             tc.tile_pool(name="mm", bufs=2, space="PSUM") as mmpool, \
             tc.tile_pool(name="tr", bufs=4, space="PSUM") as trpool:

            ident = cpool.tile([P, P], F32)
            make_identity(nc, ident[:])

            wg_ctx = tc.tile_pool(name="wg", bufs=1)
            wgpool = wg_ctx.__enter__()
            # W_gen resident: [128, 8 chunks, 1024]
            wg_t = wgpool.tile([P, 8, 2 * F], F32)
            nc.scalar.dma_start(out=wg_t[:], in_=wgen.ap().rearrange(
                "(c p) n -> p c n", p=P))

            # neigh_idx: [128, blk, k]
            idx_t = cpool.tile([P, NBLK, K], I32)
            nc.scalar.dma_start(out=idx_t[:], in_=nidx.ap().rearrange(
                "(b p) k -> p b k", p=P))

            # gamma/beta as [128, 12]
            gb_t = cpool.tile([P, 24], F32)
            nc.scalar.dma_start(out=gb_t[:, 0:12], in_=gamma.ap().rearrange(
                "(c p) -> p c", p=P))
            nc.scalar.dma_start(out=gb_t[:, 12:24], in_=beta.ap().rearrange(
                "(c p) -> p c", p=P))

            stacc = stpool.tile([P, 24], F32, tag="stacc")
            ft_blocks = []

            # ---------------- Phase 1: per seed-block ----------------
            for blk in range(NBLK):
                S = vpool.tile([P, F], F32, tag="S")
                selfv = vpool.tile([P, F], F32, tag="selfv")
                part = vpool.tile([P, F], F32, tag="part")
                for g in range(NSUB):
                    gt = gpool.tile([P, KSUB * F], F32, tag="gt")
                    for kk in range(KSUB):
                        k = g * KSUB + kk
                        nc.gpsimd.indirect_dma_start(
                            out=gt[:, kk * F:(kk + 1) * F], out_offset=None,
                            in_=table.ap(),
                            in_offset=bass.IndirectOffsetOnAxis(
                                ap=idx_t[:, blk, k:k + 1], axis=0))
                    red_view = gt[:].rearrange("p (k c) -> p c k", c=F)
                    if g == 0:
                        nc.vector.tensor_copy(out=selfv[:], in_=gt[:, 0:F])
                        nc.vector.tensor_reduce(out=S[:], in_=red_view,
                                                axis=AX.X, op=ALU.add)
                    else:
                        nc.vector.tensor_reduce(out=part[:], in_=red_view,
                                                axis=AX.X, op=ALU.add)
                        nc.vector.tensor_add(out=S[:], in0=S[:], in1=part[:])

                # agg = S/32 ; D = S - self (env_agg*31)
                aggv = vpool.tile([P, F], F32, tag="aggv")
                nc.vector.tensor_scalar_mul(aggv[:], S[:], 1.0 / K)
                nc.sync.dma_start(out=agg_v[:, blk, :], in_=aggv[:])
                Dv = vpool.tile([P, F], F32, tag="Dv")
                nc.vector.tensor_tensor(out=Dv[:], in0=S[:], in1=selfv[:],
                                        op=ALU.subtract)

                # transposes: selfT -> catT(0..3); aggT -> fT(0..3); DT
                catT = catpool.tile([P, 4, P], F32, tag="catT")
                DT = catpool.tile([P, 4, P], F32, tag="DT")
                fT = fpool.tile([P, 12, P], F32, tag="fT")
                ft_blocks.append(fT)
                for c in range(4):
                    ps = trpool.tile([P, P], F32, tag="trp")
                    nc.tensor.transpose(out=ps[:], in_=selfv[:, c * P:(c + 1) * P],
                                        identity=ident[:])
                    nc.vector.tensor_copy(out=catT[:, c, :], in_=ps[:])
                for c in range(4):
                    ps = trpool.tile([P, P], F32, tag="trp")
                    nc.tensor.transpose(out=ps[:], in_=aggv[:, c * P:(c + 1) * P],
                                        identity=ident[:])
                    nc.vector.tensor_copy(out=fT[:, c, :], in_=ps[:])
                for c in range(4):
                    ps = trpool.tile([P, P], F32, tag="trp")
                    nc.tensor.transpose(out=ps[:], in_=Dv[:, c * P:(c + 1) * P],
                                        identity=ident[:])
                    nc.vector.tensor_copy(out=DT[:, c, :], in_=ps[:])

                # gen matmul: raw[s,n] = sum_c cat[s,c] Wg[c,n]
                genv = genpool.tile([P, 2 * F], F32, tag="genv")
                for h in range(2):
                    ps = mmpool.tile([P, F], F32, tag="mmp")
                    for c in range(8):
                        lhsT = catT[:, c, :] if c < 4 else fT[:, c - 4, :]
                        nc.tensor.matmul(out=ps[:], lhsT=lhsT,
                                         rhs=wg_t[:, c, h * F:(h + 1) * F],
                                         start=(c == 0), stop=(c == 7))
                    ro = opool.tile([P, F], F32, tag="ost")
                    nc.vector.tensor_copy(out=ro[:], in_=ps[:])
                    nc.sync.dma_start(out=raw_v[:, blk, h * F:(h + 1) * F], in_=ro[:])
                    nc.scalar.activation(out=genv[:, h * F:(h + 1) * F], in_=ps[:],
                                         func=AF.Relu)
                nc.sync.dma_start(out=gen_v[:, blk, :], in_=genv[:])

                # genT -> fT(4..11)
                for c in range(8):
                    ps = trpool.tile([P, P], F32, tag="trp")
                    nc.tensor.transpose(out=ps[:], in_=genv[:, c * P:(c + 1) * P],
                                        identity=ident[:])
                    nc.vector.tensor_copy(out=fT[:, 4 + c, :], in_=ps[:])

                # env matmul: eraw = (D @ Wg[512:,:]) / 31
                for h in range(2):
                    ps = mmpool.tile([P, F], F32, tag="mmp")
                    for c in range(4):
                        nc.tensor.matmul(out=ps[:], lhsT=DT[:, c, :],
                                         rhs=wg_t[:, 4 + c, h * F:(h + 1) * F],
                                         start=(c == 0), stop=(c == 3))
                    ro = opool.tile([P, F], F32, tag="ost")
                    nc.vector.tensor_scalar_mul(ro[:], ps[:], 1.0 / (K - 1))
                    nc.sync.dma_start(out=eraw_v[:, blk, h * F:(h + 1) * F], in_=ro[:])
                    ro2 = opool.tile([P, F], F32, tag="ost")
                    nc.scalar.activation(out=ro2[:], in_=ps[:], func=AF.Relu,
                                         scale=1.0 / (K - 1))
                    nc.sync.dma_start(out=egen_v[:, blk, h * F:(h + 1) * F], in_=ro2[:])

                # BN partial stats for this block: sum and sumsq over seeds
                sq = sqpool.tile([P, 12 * P], F32, tag="sq")
                nc.vector.tensor_tensor(out=sq[:], in0=fT[:], in1=fT[:], op=ALU.mult)
                st = stpool.tile([P, 24], F32, tag="stblk")
                nc.vector.tensor_reduce(out=st[:, 0:12], in_=fT[:],
                                        axis=AX.X, op=ALU.add)
                nc.vector.tensor_reduce(out=st[:, 12:24],
                                        in_=sq[:].rearrange("p (c s) -> p c s", s=P),
                                        axis=AX.X, op=ALU.add)
                if blk == 0:
                    nc.vector.tensor_copy(out=stacc[:], in_=st[:])
                else:
                    nc.vector.tensor_add(out=stacc[:], in0=stacc[:], in1=st[:])

            wg_ctx.__exit__(None, None, None)

            # ---------------- BN stats all-reduce ----------------
            nc.sync.dma_start(out=cc_in.ap(), in_=stacc[:])
            nc.gpsimd.collective_compute(
                "AllReduce", ALU.add, replica_groups=[list(range(N_CORES))],
                ins=[cc_in.ap()], outs=[cc_out.ap()])
            stg = stpool.tile([P, 24], F32, tag="stg")
            nc.sync.dma_start(out=stg[:], in_=cc_out.ap())

            # mu = sum/B ; var = sumsq/B - mu^2 ; s = gamma/sqrt(var+eps)
            # b = beta - mu*s
            mu = stpool.tile([P, 12], F32, tag="mu")
            nc.vector.tensor_scalar_mul(mu[:], stg[:, 0:12], 1.0 / B)
            ex2 = stpool.tile([P, 12], F32, tag="ex2")
            nc.vector.tensor_scalar_mul(ex2[:], stg[:, 12:24], 1.0 / B)
            musq = stpool.tile([P, 12], F32, tag="musq")
            nc.vector.tensor_tensor(out=musq[:], in0=mu[:], in1=mu[:], op=ALU.mult)
            var = stpool.tile([P, 12], F32, tag="var")
            nc.vector.tensor_tensor(out=var[:], in0=ex2[:], in1=musq[:],
                                    op=ALU.subtract)
            eps_t = stpool.tile([P, 1], F32, tag="eps")
            nc.gpsimd.memset(eps_t[:], BN_EPS)
            sd = stpool.tile([P, 12], F32, tag="sd")
            nc.scalar.activation(out=sd[:], in_=var[:], func=AF.Sqrt, bias=eps_t[:])
            rsd = stpool.tile([P, 12], F32, tag="rsd")
            nc.vector.reciprocal(out=rsd[:], in_=sd[:])
            bn_s = stpool.tile([P, 12], F32, tag="bn_s")
            nc.vector.tensor_tensor(out=bn_s[:], in0=rsd[:], in1=gb_t[:, 0:12],
                                    op=ALU.mult)
            mus = stpool.tile([P, 12], F32, tag="mus")
            nc.vector.tensor_tensor(out=mus[:], in0=mu[:], in1=bn_s[:], op=ALU.mult)
            bn_b = stpool.tile([P, 12], F32, tag="bn_b")
            nc.vector.tensor_tensor(out=bn_b[:], in0=gb_t[:, 12:24], in1=mus[:],
                                    op=ALU.subtract)

            # ---------------- Phase 2: BN apply + final matmul ----------------
            for blk in range(NBLK):
                fT = ft_blocks[blk]
                for c in range(12):
                    nc.vector.tensor_scalar(
                        out=fT[:, c, :], in0=fT[:, c, :],
                        scalar1=bn_s[:, c:c + 1], scalar2=bn_b[:, c:c + 1],
                        op0=ALU.mult, op1=ALU.add)

            w1_ctx = tc.tile_pool(name="w1p", bufs=2)
            w1pool = w1_ctx.__enter__()
            for n in range(3):
                w1_t = w1pool.tile([P, 12, F], F32, tag="w1")
                nc.scalar.dma_start(out=w1_t[:], in_=w1.ap().rearrange(
                    "(c p) n -> p c n", p=P)[:, :, n * F:(n + 1) * F])
                for blk in range(NBLK):
                    fT = ft_blocks[blk]
                    ps = mmpool.tile([P, F], F32, tag="mmp")
                    for c in range(12):
                        nc.tensor.matmul(out=ps[:], lhsT=fT[:, c, :],
                                         rhs=w1_t[:, c, :],
                                         start=(c == 0), stop=(c == 11))
                    ro = opool.tile([P, F], F32, tag="ost")
                    nc.scalar.activation(out=ro[:], in_=ps[:], func=AF.Relu)
                    nc.sync.dma_start(out=to_v[:, blk, n * F:(n + 1) * F], in_=ro[:])
            w1_ctx.__exit__(None, None, None)

    nc.compile()
    return nc


def _get_nc():
    if "nc" not in _CACHE:
        _CACHE["nc"] = _build()
    return _CACHE["nc"]


def _make_in_maps(feat_table, W_gen, weight1, gamma, beta, neigh_idx):
    feat_table = np.ascontiguousarray(feat_table, dtype=np.float32)
    W_gen = np.ascontiguousarray(W_gen, dtype=np.float32)
    weight1 = np.ascontiguousarray(weight1, dtype=np.float32)
    gamma = np.ascontiguousarray(gamma, dtype=np.float32)
    beta = np.ascontiguousarray(beta, dtype=np.float32)
    neigh_idx = np.ascontiguousarray(neigh_idx, dtype=np.int32)
    maps = []
    for c in range(N_CORES):
        sl = slice(c * BL, (c + 1) * BL)
        maps.append({
            "feat_table": feat_table, "W_gen": W_gen, "weight1": weight1,
            "gamma": gamma, "beta": beta, "neigh_idx": neigh_idx[sl],
        })
    return maps


def _run(inputs, trace=False, tmpdir=None):
    from concourse.bass_utils import run_bass_kernel_spmd
    nc = _get_nc()
    in_maps = _make_in_maps(
        inputs["feat_table"], inputs["W_gen"], inputs["weight1"],
        inputs["gamma"], inputs["beta"], inputs["neigh_idx"])
    res = run_bass_kernel_spmd(nc, in_maps, core_ids=list(range(N_CORES)),
                               trace=trace, tmpdir=tmpdir)
    outs = res.results
    cat = lambda name: np.concatenate([outs[c][name] for c in range(N_CORES)], axis=0)
    result = (cat("agg"), cat("to"), cat("gen"), cat("raw"),
              cat("egen"), cat("eraw"))
    return result, res


def kernel(feat_table, W_gen, weight1, gamma, beta, nodes, neigh_idx):
    result, _ = _run({
        "feat_table": feat_table, "W_gen": W_gen, "weight1": weight1,
        "gamma": gamma, "beta": beta, "neigh_idx": neigh_idx})
    return result


# revision 6
# speedup vs baseline: 1.3483x; 1.3483x over previous
"""Trainium2 Bass kernel for nn_IntraAgg (GraphSAGE-style intra-aggregation).

Strategy: data-parallel over the 8192 seed nodes across 8 NeuronCores
(1024 seeds/core); feat_table and weights replicated. Per core:
  - indirect-DMA gather of the 32 neighbor rows per seed (2KB rows)
  - neighbor-sum via strided VectorE reduce
  - gen / env matmuls on TensorE (activations transposed on-chip via PE)
  - BatchNorm stats via per-core partial sums + 12KB AllReduce
  - final 1536x1536 matmul + relu
All fp32.
"""
import sys
sys.path.insert(0, '/opt/trn_rl_repo')

import numpy as np

import concourse.bass as bass
import concourse.mybir as mybir
from concourse import bacc
from concourse.tile import TileContext
from concourse.masks import make_identity

F32 = mybir.dt.float32
BF16 = mybir.dt.bfloat16
I32 = mybir.dt.int32
AF = mybir.ActivationFunctionType
ALU = mybir.AluOpType
AX = mybir.AxisListType

N_CORES = 8
N_NODES = 100000
F = 512            # feat dim
K = 32             # neighbors (col 0 = self)
B = 8192           # global batch
BL = B // N_CORES  # 1024 seeds per core
P = 128
NBLK = BL // P     # 8 seed-blocks per core
BN_EPS = 1e-5
KSUB = 8           # indices per sub-gather
NSUB = K // KSUB   # 4 sub-gathers per block

_CACHE = {}


def _build():
    nc = bacc.Bacc("TRN2", target_bir_lowering=False, debug=False,
                   num_devices=N_CORES)

    table = nc.declare_dram_parameter("feat_table", [N_NODES, F], F32, isOutput=False)
    wgen = nc.declare_dram_parameter("W_gen", [2 * F, 2 * F], BF16, isOutput=False)
    w1 = nc.declare_dram_parameter("weight1", [3 * F, 3 * F], BF16, isOutput=False)
    gamma = nc.declare_dram_parameter("gamma", [3 * F], F32, isOutput=False)
    beta = nc.declare_dram_parameter("beta", [3 * F], F32, isOutput=False)
    nidx = nc.declare_dram_parameter("neigh_idx", [BL, K], I32, isOutput=False)

    agg_o = nc.declare_dram_parameter("agg", [BL, F], F32, isOutput=True)
    to_o = nc.declare_dram_parameter("to", [BL, 3 * F], F32, isOutput=True)
    gen_o = nc.declare_dram_parameter("gen", [BL, 2 * F], F32, isOutput=True)
    raw_o = nc.declare_dram_parameter("raw", [BL, 2 * F], F32, isOutput=True)
    egen_o = nc.declare_dram_parameter("egen", [BL, 2 * F], F32, isOutput=True)
    eraw_o = nc.declare_dram_parameter("eraw", [BL, 2 * F], F32, isOutput=True)

    cc_in = nc.dram_tensor("cc_in", [P, 24], F32)
    cc_out = nc.dram_tensor("cc_out", [P, 24], F32, addr_space="Shared")

    # blocked views: seed s = blk*128 + p
    def blocked(h, nch):
        return h.ap().rearrange("(b p) c -> p b c", p=P)

    agg_v = blocked(agg_o, F)
    to_v = blocked(to_o, 3 * F)
    gen_v = blocked(gen_o, 2 * F)
    raw_v = blocked(raw_o, 2 * F)
    egen_v = blocked(egen_o, 2 * F)
    eraw_v = blocked(eraw_o, 2 * F)

    with TileContext(nc) as tc:
        with tc.tile_pool(name="const", bufs=1) as cpool, \
             tc.tile_pool(name="gath", bufs=3) as gpool, \
             tc.tile_pool(name="red", bufs=2) as rpool, \
             tc.tile_pool(name="vec", bufs=2) as vpool, \
             tc.tile_pool(name="cat", bufs=2) as catpool, \
             tc.tile_pool(name="ft", bufs=NBLK) as fpool, \
             tc.tile_pool(name="ost", bufs=4) as opool, \
             tc.tile_pool(name="genp", bufs=2) as genpool, \
             tc.tile_pool(name="stat", bufs=2) as stpool, \
             tc.tile_pool(name="sq", bufs=1) as sqpool, \
             tc.tile_pool(name="mm", bufs=2, space="PSUM") as mmpool, \
             tc.tile_pool(name="tr", bufs=4, space="PSUM") as trpool:

            ident = cpool.tile([P, P], F32)
            make_identity(nc, ident[:])

            wg_ctx = tc.tile_pool(name="wg", bufs=1)
            wgpool = wg_ctx.__enter__()
            # W_gen resident: [128, 8 chunks, 1024]
            wg_t = wgpool.tile([P, 8, 2 * F], BF16)
            nc.scalar.dma_start(out=wg_t[:], in_=wgen.ap().rearrange(
                "(c p) n -> p c n", p=P))

            # neigh_idx: [128, blk, k]
            idx_t = cpool.tile([P, NBLK, K], I32)
            nc.scalar.dma_start(out=idx_t[:], in_=nidx.ap().rearrange(
                "(b p) k -> p b k", p=P))

            # gamma/beta as [128, 12]
            gb_t = cpool.tile([P, 24], F32)
            nc.scalar.dma_start(out=gb_t[:, 0:12], in_=gamma.ap().rearrange(
                "(c p) -> p c", p=P))
            nc.scalar.dma_start(out=gb_t[:, 12:24], in_=beta.ap().rearrange(
                "(c p) -> p c", p=P))

            stacc = stpool.tile([P, 24], F32, tag="stacc")
            ft_blocks = []

            # ---------------- Phase 1: per seed-block ----------------
            for blk in range(NBLK):
                S = vpool.tile([P, F], F32, tag="S")
                selfv = vpool.tile([P, F], F32, tag="selfv")
                part = vpool.tile([P, F], F32, tag="part")
                for g in range(NSUB):
                    gt = gpool.tile([P, KSUB * F], F32, tag="gt")
                    for kk in range(KSUB):
                        k = g * KSUB + kk
                        nc.gpsimd.indirect_dma_start(
                            out=gt[:, kk * F:(kk + 1) * F], out_offset=None,
                            in_=table.ap(),
                            in_offset=bass.IndirectOffsetOnAxis(
                                ap=idx_t[:, blk, k:k + 1], axis=0))
                    if g == 0:
                        nc.vector.tensor_copy(out=selfv[:], in_=gt[:, 0:F])
                    t8 = rpool.tile([P, 4 * F], F32, tag="t8")
                    nc.vector.tensor_add(out=t8[:], in0=gt[:, 0:4 * F],
                                         in1=gt[:, 4 * F:8 * F])
                    t4 = rpool.tile([P, 2 * F], F32, tag="t4")
                    nc.vector.tensor_add(out=t4[:], in0=t8[:, 0:2 * F],
                                         in1=t8[:, 2 * F:4 * F])
                    if g == 0:
                        nc.vector.tensor_add(out=S[:], in0=t4[:, 0:F],
                                             in1=t4[:, F:2 * F])
                    else:
                        nc.vector.tensor_add(out=part[:], in0=t4[:, 0:F],
                                             in1=t4[:, F:2 * F])
                        nc.vector.tensor_add(out=S[:], in0=S[:], in1=part[:])

                # agg = S/32 ; D = S - self (env_agg*31)
                aggv = vpool.tile([P, F], F32, tag="aggv")
                nc.vector.tensor_scalar_mul(aggv[:], S[:], 1.0 / K)
                nc.sync.dma_start(out=agg_v[:, blk, :], in_=aggv[:])
                Dv = vpool.tile([P, F], F32, tag="Dv")
                nc.vector.tensor_tensor(out=Dv[:], in0=S[:], in1=selfv[:],
                                        op=ALU.subtract)

                # transposes: selfT -> catT(0..3); aggT -> fT(0..3); DT
                catT = catpool.tile([P, 4, P], BF16, tag="catT")
                DT = catpool.tile([P, 4, P], BF16, tag="DT")
                fT = fpool.tile([P, 12, P], BF16, tag="fT")
                ft_blocks.append(fT)
                for c in range(4):
                    ps = trpool.tile([P, P], F32, tag="trp")
                    nc.tensor.transpose(out=ps[:], in_=selfv[:, c * P:(c + 1) * P],
                                        identity=ident[:])
                    nc.vector.tensor_copy(out=catT[:, c, :], in_=ps[:])
                for c in range(4):
                    ps = trpool.tile([P, P], F32, tag="trp")
                    nc.tensor.transpose(out=ps[:], in_=aggv[:, c * P:(c + 1) * P],
                                        identity=ident[:])
                    nc.vector.tensor_copy(out=fT[:, c, :], in_=ps[:])
                for c in range(4):
                    ps = trpool.tile([P, P], F32, tag="trp")
                    nc.tensor.transpose(out=ps[:], in_=Dv[:, c * P:(c + 1) * P],
                                        identity=ident[:])
                    nc.vector.tensor_copy(out=DT[:, c, :], in_=ps[:])

                # gen matmul: raw[s,n] = sum_c cat[s,c] Wg[c,n]
                genv = genpool.tile([P, 2 * F], F32, tag="genv")
                for h in range(2):
                    ps = mmpool.tile([P, F], F32, tag="mmp")
                    for c in range(8):
                        lhsT = catT[:, c, :] if c < 4 else fT[:, c - 4, :]
                        nc.tensor.matmul(out=ps[:], lhsT=lhsT,
                                         rhs=wg_t[:, c, h * F:(h + 1) * F],
                                         start=(c == 0), stop=(c == 7))
                    ro = opool.tile([P, F], F32, tag="ost")
                    nc.vector.tensor_copy(out=ro[:], in_=ps[:])
                    nc.sync.dma_start(out=raw_v[:, blk, h * F:(h + 1) * F], in_=ro[:])
                    nc.scalar.activation(out=genv[:, h * F:(h + 1) * F], in_=ps[:],
                                         func=AF.Relu)
                nc.sync.dma_start(out=gen_v[:, blk, :], in_=genv[:])

                # genT -> fT(4..11)
                for c in range(8):
                    ps = trpool.tile([P, P], F32, tag="trp")
                    nc.tensor.transpose(out=ps[:], in_=genv[:, c * P:(c + 1) * P],
                                        identity=ident[:])
                    nc.vector.tensor_copy(out=fT[:, 4 + c, :], in_=ps[:])

                # env matmul: eraw = (D @ Wg[512:,:]) / 31
                for h in range(2):
                    ps = mmpool.tile([P, F], F32, tag="mmp")
                    for c in range(4):
                        nc.tensor.matmul(out=ps[:], lhsT=DT[:, c, :],
                                         rhs=wg_t[:, 4 + c, h * F:(h + 1) * F],
                                         start=(c == 0), stop=(c == 3))
                    ro = opool.tile([P, F], F32, tag="ost")
                    nc.vector.tensor_scalar_mul(ro[:], ps[:], 1.0 / (K - 1))
                    nc.sync.dma_start(out=eraw_v[:, blk, h * F:(h + 1) * F], in_=ro[:])
                    ro2 = opool.tile([P, F], F32, tag="ost")
                    nc.scalar.activation(out=ro2[:], in_=ps[:], func=AF.Relu,
                                         scale=1.0 / (K - 1))
                    nc.sync.dma_start(out=egen_v[:, blk, h * F:(h + 1) * F], in_=ro2[:])

                # BN partial stats for this block: sum and sumsq over seeds
                sq = sqpool.tile([P, 12 * P], F32, tag="sq")
                nc.vector.tensor_tensor(out=sq[:], in0=fT[:], in1=fT[:], op=ALU.mult)
                st = stpool.tile([P, 24], F32, tag="stblk")
                nc.vector.tensor_reduce(out=st[:, 0:12], in_=fT[:],
                                        axis=AX.X, op=ALU.add)
                nc.vector.tensor_reduce(out=st[:, 12:24],
                                        in_=sq[:].rearrange("p (c s) -> p c s", s=P),
                                        axis=AX.X, op=ALU.add)
                if blk == 0:
                    nc.vector.tensor_copy(out=stacc[:], in_=st[:])
                else:
                    nc.vector.tensor_add(out=stacc[:], in0=stacc[:], in1=st[:])

            wg_ctx.__exit__(None, None, None)

            # ---------------- BN stats all-reduce ----------------
            nc.sync.dma_start(out=cc_in.ap(), in_=stacc[:])
            nc.gpsimd.collective_compute(
                "AllReduce", ALU.add, replica_groups=[list(range(N_CORES))],
                ins=[cc_in.ap()], outs=[cc_out.ap()])
            stg = stpool.tile([P, 24], F32, tag="stg")
            nc.sync.dma_start(out=stg[:], in_=cc_out.ap())

            # mu = sum/B ; var = sumsq/B - mu^2 ; s = gamma/sqrt(var+eps)
            # b = beta - mu*s
            mu = stpool.tile([P, 12], F32, tag="mu")
            nc.vector.tensor_scalar_mul(mu[:], stg[:, 0:12], 1.0 / B)
            ex2 = stpool.tile([P, 12], F32, tag="ex2")
            nc.vector.tensor_scalar_mul(ex2[:], stg[:, 12:24], 1.0 / B)
            musq = stpool.tile([P, 12], F32, tag="musq")
            nc.vector.tensor_tensor(out=musq[:], in0=mu[:], in1=mu[:], op=ALU.mult)
            var = stpool.tile([P, 12], F32, tag="var")
            nc.vector.tensor_tensor(out=var[:], in0=ex2[:], in1=musq[:],
                                    op=ALU.subtract)
            eps_t = stpool.tile([P, 1], F32, tag="eps")
            nc.gpsimd.memset(eps_t[:], BN_EPS)
            sd = stpool.tile([P, 12], F32, tag="sd")
            nc.scalar.activation(out=sd[:], in_=var[:], func=AF.Sqrt, bias=eps_t[:])
            rsd = stpool.tile([P, 12], F32, tag="rsd")
            nc.vector.reciprocal(out=rsd[:], in_=sd[:])
            bn_s = stpool.tile([P, 12], F32, tag="bn_s")
            nc.vector.tensor_tensor(out=bn_s[:], in0=rsd[:], in1=gb_t[:, 0:12],
                                    op=ALU.mult)
            mus = stpool.tile([P, 12], F32, tag="mus")
            nc.vector.tensor_tensor(out=mus[:], in0=mu[:], in1=bn_s[:], op=ALU.mult)
            bn_b = stpool.tile([P, 12], F32, tag="bn_b")
            nc.vector.tensor_tensor(out=bn_b[:], in0=gb_t[:, 12:24], in1=mus[:],
                                    op=ALU.subtract)

            # ---------------- Phase 2: BN apply + final matmul ----------------
            for blk in range(NBLK):
                fT = ft_blocks[blk]
                for c in range(12):
                    nc.vector.tensor_scalar(
                        out=fT[:, c, :], in0=fT[:, c, :],
                        scalar1=bn_s[:, c:c + 1], scalar2=bn_b[:, c:c + 1],
                        op0=ALU.mult, op1=ALU.add)

            w1_ctx = tc.tile_pool(name="w1p", bufs=2)
            w1pool = w1_ctx.__enter__()
            for n in range(3):
                w1_t = w1pool.tile([P, 12, F], BF16, tag="w1")
                nc.scalar.dma_start(out=w1_t[:], in_=w1.ap().rearrange(
                    "(c p) n -> p c n", p=P)[:, :, n * F:(n + 1) * F])
                for blk in range(NBLK):
                    fT = ft_blocks[blk]
                    ps = mmpool.tile([P, F], F32, tag="mmp")
                    for c in range(12):
                        nc.tensor.matmul(out=ps[:], lhsT=fT[:, c, :],
                                         rhs=w1_t[:, c, :],
                                         start=(c == 0), stop=(c == 11))
                    ro = opool.tile([P, F], F32, tag="ost")
                    nc.scalar.activation(out=ro[:], in_=ps[:], func=AF.Relu)
                    nc.sync.dma_start(out=to_v[:, blk, n * F:(n + 1) * F], in_=ro[:])
            w1_ctx.__exit__(None, None, None)

    nc.compile()
    return nc


def _get_nc():
    if "nc" not in _CACHE:
        _CACHE["nc"] = _build()
    return _CACHE["nc"]


def _make_in_maps(feat_table, W_gen, weight1, gamma, beta, neigh_idx):
    import ml_dtypes
    feat_table = np.ascontiguousarray(feat_table, dtype=np.float32)
    W_gen = np.ascontiguousarray(W_gen).astype(ml_dtypes.bfloat16)
    weight1 = np.ascontiguousarray(weight1).astype(ml_dtypes.bfloat16)
    gamma = np.ascontiguousarray(gamma, dtype=np.float32)
    beta = np.ascontiguousarray(beta, dtype=np.float32)
    neigh_idx = np.ascontiguousarray(neigh_idx, dtype=np.int32)
    maps = []
    for c in range(N_CORES):
        sl = slice(c * BL, (c + 1) * BL)
        maps.append({
            "feat_table": feat_table, "W_gen": W_gen, "weight1": weight1,
            "gamma": gamma, "beta": beta, "neigh_idx": neigh_idx[sl],
        })
    return maps


def _run(inputs, trace=False, tmpdir=None):
    from concourse.bass_utils import run_bass_kernel_spmd
    nc = _get_nc()
    in_maps = _make_in_maps(
        inputs["feat_table"], inputs["W_gen"], inputs["weight1"],
        inputs["gamma"], inputs["beta"], inputs["neigh_idx"])
    res = run_bass_kernel_spmd(nc, in_maps, core_ids=list(range(N_CORES)),
                               trace=trace, tmpdir=tmpdir)
    outs = res.results
    cat = lambda name: np.concatenate([outs[c][name] for c in range(N_CORES)], axis=0)
    result = (cat("agg"), cat("to"), cat("gen"), cat("raw"),
              cat("egen"), cat("eraw"))
    return result, res


def kernel(feat_table, W_gen, weight1, gamma, beta, nodes, neigh_idx):
    result, _ = _run({
        "feat_table": feat_table, "W_gen": W_gen, "weight1": weight1,
        "gamma": gamma, "beta": beta, "neigh_idx": neigh_idx})
    return result


# revision 7
# speedup vs baseline: 1.3654x; 1.0126x over previous
"""Trainium2 Bass kernel for nn_IntraAgg (GraphSAGE-style intra-aggregation).

Strategy: data-parallel over the 8192 seed nodes across 8 NeuronCores
(1024 seeds/core); feat_table and weights replicated. Per core:
  - indirect-DMA gather of the 32 neighbor rows per seed (2KB rows)
  - neighbor-sum via strided VectorE reduce
  - gen / env matmuls on TensorE (activations transposed on-chip via PE)
  - BatchNorm stats via per-core partial sums + 12KB AllReduce
  - final 1536x1536 matmul + relu
All fp32.
"""
import sys
sys.path.insert(0, '/opt/trn_rl_repo')

import numpy as np

import concourse.bass as bass
import concourse.mybir as mybir
from concourse import bacc
from concourse.tile import TileContext
from concourse.masks import make_identity

F32 = mybir.dt.float32
BF16 = mybir.dt.bfloat16
I32 = mybir.dt.int32
AF = mybir.ActivationFunctionType
ALU = mybir.AluOpType
AX = mybir.AxisListType

N_CORES = 8
N_NODES = 100000
F = 512            # feat dim
K = 32             # neighbors (col 0 = self)
B = 8192           # global batch
BL = B // N_CORES  # 1024 seeds per core
P = 128
NBLK = BL // P     # 8 seed-blocks per core
BN_EPS = 1e-5
KSUB = 8           # indices per sub-gather
NSUB = K // KSUB   # 4 sub-gathers per block

_CACHE = {}


def _build():
    nc = bacc.Bacc("TRN2", target_bir_lowering=False, debug=False,
                   num_devices=N_CORES)

    table = nc.declare_dram_parameter("feat_table", [N_NODES, F], F32, isOutput=False)
    wgen = nc.declare_dram_parameter("W_gen", [2 * F, 2 * F], BF16, isOutput=False)
    w1 = nc.declare_dram_parameter("weight1", [3 * F, 3 * F], BF16, isOutput=False)
    gamma = nc.declare_dram_parameter("gamma", [3 * F], F32, isOutput=False)
    beta = nc.declare_dram_parameter("beta", [3 * F], F32, isOutput=False)
    nidx = nc.declare_dram_parameter("neigh_idx", [BL, K], I32, isOutput=False)

    agg_o = nc.declare_dram_parameter("agg", [BL, F], F32, isOutput=True)
    to_o = nc.declare_dram_parameter("to", [BL, 3 * F], F32, isOutput=True)
    gen_o = nc.declare_dram_parameter("gen", [BL, 2 * F], F32, isOutput=True)
    raw_o = nc.declare_dram_parameter("raw", [BL, 2 * F], F32, isOutput=True)
    egen_o = nc.declare_dram_parameter("egen", [BL, 2 * F], F32, isOutput=True)
    eraw_o = nc.declare_dram_parameter("eraw", [BL, 2 * F], F32, isOutput=True)

    cc_in = nc.dram_tensor("cc_in", [P, 24], F32)
    cc_out = nc.dram_tensor("cc_out", [P, 24], F32, addr_space="Shared")

    # blocked views: seed s = blk*128 + p
    def blocked(h, nch):
        return h.ap().rearrange("(b p) c -> p b c", p=P)

    agg_v = blocked(agg_o, F)
    to_v = blocked(to_o, 3 * F)
    gen_v = blocked(gen_o, 2 * F)
    raw_v = blocked(raw_o, 2 * F)
    egen_v = blocked(egen_o, 2 * F)
    eraw_v = blocked(eraw_o, 2 * F)

    with TileContext(nc) as tc:
        with tc.tile_pool(name="const", bufs=1) as cpool, \
             tc.tile_pool(name="gath", bufs=3) as gpool, \
             tc.tile_pool(name="vec", bufs=2) as vpool, \
             tc.tile_pool(name="cat", bufs=2) as catpool, \
             tc.tile_pool(name="ft", bufs=NBLK) as fpool, \
             tc.tile_pool(name="ost", bufs=4) as opool, \
             tc.tile_pool(name="genp", bufs=2) as genpool, \
             tc.tile_pool(name="stat", bufs=2) as stpool, \
             tc.tile_pool(name="sq", bufs=1) as sqpool, \
             tc.tile_pool(name="mm", bufs=4, space="PSUM") as mmpool, \
             tc.tile_pool(name="tr", bufs=4, space="PSUM") as trpool:

            ident = cpool.tile([P, P], F32)
            make_identity(nc, ident[:])

            # neigh_idx first: gathers depend on it
            idx_t = cpool.tile([P, NBLK, K], I32)
            nc.scalar.dma_start(out=idx_t[:], in_=nidx.ap().rearrange(
                "(b p) k -> p b k", p=P))

            wg_ctx = tc.tile_pool(name="wg", bufs=1)
            wgpool = wg_ctx.__enter__()
            # W_gen resident: [128, 8 chunks, 1024]
            wg_t = wgpool.tile([P, 8, 2 * F], BF16)
            nc.scalar.dma_start(out=wg_t[:], in_=wgen.ap().rearrange(
                "(c p) n -> p c n", p=P))

            # weight1 resident (bf16, 36KB/partition), prefetched up front
            w1_t = cpool.tile([P, 12, 3 * F], BF16)
            nc.sync.dma_start(out=w1_t[:], in_=w1.ap().rearrange(
                "(c p) n -> p c n", p=P))

            # gamma/beta as [128, 12]
            gb_t = cpool.tile([P, 24], F32)
            nc.scalar.dma_start(out=gb_t[:, 0:12], in_=gamma.ap().rearrange(
                "(c p) -> p c", p=P))
            nc.scalar.dma_start(out=gb_t[:, 12:24], in_=beta.ap().rearrange(
                "(c p) -> p c", p=P))

            stacc = stpool.tile([P, 24], F32, tag="stacc")
            ft_blocks = []

            # ---------------- Phase 1: per seed-block ----------------
            for blk in range(NBLK):
                S = vpool.tile([P, F], F32, tag="S")
                selfv = vpool.tile([P, F], F32, tag="selfv")
                part = vpool.tile([P, F], F32, tag="part")
                for g in range(NSUB):
                    gt = gpool.tile([P, KSUB * F], F32, tag="gt")
                    for kk in range(KSUB):
                        k = g * KSUB + kk
                        nc.gpsimd.indirect_dma_start(
                            out=gt[:, kk * F:(kk + 1) * F], out_offset=None,
                            in_=table.ap(),
                            in_offset=bass.IndirectOffsetOnAxis(
                                ap=idx_t[:, blk, k:k + 1], axis=0))
                    if g == 0:
                        nc.scalar.activation(out=selfv[:], in_=gt[:, 0:F],
                                             func=AF.Copy)
                    nc.vector.tensor_add(out=gt[:, 0:4 * F], in0=gt[:, 0:4 * F],
                                         in1=gt[:, 4 * F:8 * F])
                    nc.vector.tensor_add(out=gt[:, 0:2 * F], in0=gt[:, 0:2 * F],
                                         in1=gt[:, 2 * F:4 * F])
                    if g == 0:
                        nc.vector.tensor_add(out=S[:], in0=gt[:, 0:F],
                                             in1=gt[:, F:2 * F])
                    else:
                        nc.vector.tensor_add(out=part[:], in0=gt[:, 0:F],
                                             in1=gt[:, F:2 * F])
                        nc.vector.tensor_add(out=S[:], in0=S[:], in1=part[:])

                # agg = S/32 ; D = S - self (env_agg*31)
                aggv = vpool.tile([P, F], F32, tag="aggv")
                nc.vector.tensor_scalar_mul(aggv[:], S[:], 1.0 / K)
                nc.sync.dma_start(out=agg_v[:, blk, :], in_=aggv[:])
                Dv = vpool.tile([P, F], F32, tag="Dv")
                nc.vector.tensor_tensor(out=Dv[:], in0=S[:], in1=selfv[:],
                                        op=ALU.subtract)

                # transposes: selfT -> catT(0..3); aggT -> fT(0..3); DT
                catT = catpool.tile([P, 4, P], BF16, tag="catT")
                DT = catpool.tile([P, 4, P], BF16, tag="DT")
                fT = fpool.tile([P, 12, P], BF16, tag="fT")
                ft_blocks.append(fT)
                for c in range(4):
                    ps = trpool.tile([P, P], F32, tag="trp")
                    nc.tensor.transpose(out=ps[:], in_=selfv[:, c * P:(c + 1) * P],
                                        identity=ident[:])
                    nc.vector.tensor_copy(out=catT[:, c, :], in_=ps[:])
                for c in range(4):
                    ps = trpool.tile([P, P], F32, tag="trp")
                    nc.tensor.transpose(out=ps[:], in_=aggv[:, c * P:(c + 1) * P],
                                        identity=ident[:])
                    nc.vector.tensor_copy(out=fT[:, c, :], in_=ps[:])
                for c in range(4):
                    ps = trpool.tile([P, P], F32, tag="trp")
                    nc.tensor.transpose(out=ps[:], in_=Dv[:, c * P:(c + 1) * P],
                                        identity=ident[:])
                    nc.vector.tensor_copy(out=DT[:, c, :], in_=ps[:])

                # gen matmul: raw[s,n] = sum_c cat[s,c] Wg[c,n]
                genv = genpool.tile([P, 2 * F], F32, tag="genv")
                for h in range(2):
                    ps = mmpool.tile([P, F], F32, tag="mmp")
                    for c in range(8):
                        lhsT = catT[:, c, :] if c < 4 else fT[:, c - 4, :]
                        nc.tensor.matmul(out=ps[:], lhsT=lhsT,
                                         rhs=wg_t[:, c, h * F:(h + 1) * F],
                                         start=(c == 0), stop=(c == 7))
                    ro = opool.tile([P, F], F32, tag="ost")
                    nc.vector.tensor_copy(out=ro[:], in_=ps[:])
                    nc.sync.dma_start(out=raw_v[:, blk, h * F:(h + 1) * F], in_=ro[:])
                    nc.scalar.activation(out=genv[:, h * F:(h + 1) * F], in_=ps[:],
                                         func=AF.Relu)
                nc.sync.dma_start(out=gen_v[:, blk, :], in_=genv[:])

                # genT -> fT(4..11)
                for c in range(8):
                    ps = trpool.tile([P, P], F32, tag="trp")
                    nc.tensor.transpose(out=ps[:], in_=genv[:, c * P:(c + 1) * P],
                                        identity=ident[:])
                    nc.vector.tensor_copy(out=fT[:, 4 + c, :], in_=ps[:])

                # env matmul: eraw = (D @ Wg[512:,:]) / 31
                for h in range(2):
                    ps = mmpool.tile([P, F], F32, tag="mmp")
                    for c in range(4):
                        nc.tensor.matmul(out=ps[:], lhsT=DT[:, c, :],
                                         rhs=wg_t[:, 4 + c, h * F:(h + 1) * F],
                                         start=(c == 0), stop=(c == 3))
                    ro = opool.tile([P, F], F32, tag="ost")
                    nc.vector.tensor_scalar_mul(ro[:], ps[:], 1.0 / (K - 1))
                    nc.sync.dma_start(out=eraw_v[:, blk, h * F:(h + 1) * F], in_=ro[:])
                    ro2 = opool.tile([P, F], F32, tag="ost")
                    nc.scalar.activation(out=ro2[:], in_=ps[:], func=AF.Relu,
                                         scale=1.0 / (K - 1))
                    nc.sync.dma_start(out=egen_v[:, blk, h * F:(h + 1) * F], in_=ro2[:])

                # BN partial stats for this block: sum and sumsq over seeds
                sq = sqpool.tile([P, 12 * P], F32, tag="sq")
                nc.vector.tensor_tensor(out=sq[:], in0=fT[:], in1=fT[:], op=ALU.mult)
                st = stpool.tile([P, 24], F32, tag="stblk")
                nc.vector.tensor_reduce(out=st[:, 0:12], in_=fT[:],
                                        axis=AX.X, op=ALU.add)
                nc.vector.tensor_reduce(out=st[:, 12:24],
                                        in_=sq[:].rearrange("p (c s) -> p c s", s=P),
                                        axis=AX.X, op=ALU.add)
                if blk == 0:
                    nc.vector.tensor_copy(out=stacc[:], in_=st[:])
                else:
                    nc.vector.tensor_add(out=stacc[:], in0=stacc[:], in1=st[:])

            wg_ctx.__exit__(None, None, None)

            # ---------------- BN stats all-reduce ----------------
            nc.sync.dma_start(out=cc_in.ap(), in_=stacc[:])
            nc.gpsimd.collective_compute(
                "AllReduce", ALU.add, replica_groups=[list(range(N_CORES))],
                ins=[cc_in.ap()], outs=[cc_out.ap()])
            stg = stpool.tile([P, 24], F32, tag="stg")
            nc.sync.dma_start(out=stg[:], in_=cc_out.ap())

            # mu = sum/B ; var = sumsq/B - mu^2 ; s = gamma/sqrt(var+eps)
            # b = beta - mu*s
            mu = stpool.tile([P, 12], F32, tag="mu")
            nc.vector.tensor_scalar_mul(mu[:], stg[:, 0:12], 1.0 / B)
            ex2 = stpool.tile([P, 12], F32, tag="ex2")
            nc.vector.tensor_scalar_mul(ex2[:], stg[:, 12:24], 1.0 / B)
            musq = stpool.tile([P, 12], F32, tag="musq")
            nc.vector.tensor_tensor(out=musq[:], in0=mu[:], in1=mu[:], op=ALU.mult)
            var = stpool.tile([P, 12], F32, tag="var")
            nc.vector.tensor_tensor(out=var[:], in0=ex2[:], in1=musq[:],
                                    op=ALU.subtract)
            eps_t = stpool.tile([P, 1], F32, tag="eps")
            nc.gpsimd.memset(eps_t[:], BN_EPS)
            sd = stpool.tile([P, 12], F32, tag="sd")
            nc.scalar.activation(out=sd[:], in_=var[:], func=AF.Sqrt, bias=eps_t[:])
            rsd = stpool.tile([P, 12], F32, tag="rsd")
            nc.vector.reciprocal(out=rsd[:], in_=sd[:])
            bn_s = stpool.tile([P, 12], F32, tag="bn_s")
            nc.vector.tensor_tensor(out=bn_s[:], in0=rsd[:], in1=gb_t[:, 0:12],
                                    op=ALU.mult)
            mus = stpool.tile([P, 12], F32, tag="mus")
            nc.vector.tensor_tensor(out=mus[:], in0=mu[:], in1=bn_s[:], op=ALU.mult)
            bn_b = stpool.tile([P, 12], F32, tag="bn_b")
            nc.vector.tensor_tensor(out=bn_b[:], in0=gb_t[:, 12:24], in1=mus[:],
                                    op=ALU.subtract)

            # ---------------- Phase 2: BN apply + final matmul ----------------
            for blk in range(NBLK):
                fT = ft_blocks[blk]
                for c in range(12):
                    nc.vector.tensor_scalar(
                        out=fT[:, c, :], in0=fT[:, c, :],
                        scalar1=bn_s[:, c:c + 1], scalar2=bn_b[:, c:c + 1],
                        op0=ALU.mult, op1=ALU.add)

            for n in range(3):
                for blk in range(NBLK):
                    fT = ft_blocks[blk]
                    ps = mmpool.tile([P, F], F32, tag="mmp")
                    for c in range(12):
                        nc.tensor.matmul(out=ps[:], lhsT=fT[:, c, :],
                                         rhs=w1_t[:, c, n * F:(n + 1) * F],
                                         start=(c == 0), stop=(c == 11))
                    ro = opool.tile([P, F], F32, tag="ost")
                    nc.scalar.activation(out=ro[:], in_=ps[:], func=AF.Relu)
                    nc.sync.dma_start(out=to_v[:, blk, n * F:(n + 1) * F], in_=ro[:])

    nc.compile()
    return nc


def _get_nc():
    if "nc" not in _CACHE:
        _CACHE["nc"] = _build()
    return _CACHE["nc"]


def _make_in_maps(feat_table, W_gen, weight1, gamma, beta, neigh_idx):
    import ml_dtypes
    feat_table = np.ascontiguousarray(feat_table, dtype=np.float32)
    W_gen = np.ascontiguousarray(W_gen).astype(ml_dtypes.bfloat16)
    weight1 = np.ascontiguousarray(weight1).astype(ml_dtypes.bfloat16)
    gamma = np.ascontiguousarray(gamma, dtype=np.float32)
    beta = np.ascontiguousarray(beta, dtype=np.float32)
    neigh_idx = np.ascontiguousarray(neigh_idx, dtype=np.int32)
    maps = []
    for c in range(N_CORES):
        sl = slice(c * BL, (c + 1) * BL)
        maps.append({
            "feat_table": feat_table, "W_gen": W_gen, "weight1": weight1,
            "gamma": gamma, "beta": beta, "neigh_idx": neigh_idx[sl],
        })
    return maps


def _run(inputs, trace=False, tmpdir=None):
    from concourse.bass_utils import run_bass_kernel_spmd
    nc = _get_nc()
    in_maps = _make_in_maps(
        inputs["feat_table"], inputs["W_gen"], inputs["weight1"],
        inputs["gamma"], inputs["beta"], inputs["neigh_idx"])
    res = run_bass_kernel_spmd(nc, in_maps, core_ids=list(range(N_CORES)),
                               trace=trace, tmpdir=tmpdir)
    outs = res.results
    cat = lambda name: np.concatenate([outs[c][name] for c in range(N_CORES)], axis=0)
    result = (cat("agg"), cat("to"), cat("gen"), cat("raw"),
              cat("egen"), cat("eraw"))
    return result, res


def kernel(feat_table, W_gen, weight1, gamma, beta, nodes, neigh_idx):
    result, _ = _run({
        "feat_table": feat_table, "W_gen": W_gen, "weight1": weight1,
        "gamma": gamma, "beta": beta, "neigh_idx": neigh_idx})
    return result
